# revision 1
# baseline (speedup 1.0000x reference)
"""Multi-head self-attention (B=2, S=2048, E=1024, H=16, causal) on 8 trn2 cores.

Sharding: core c handles batch b = c // 4 and heads [4*(c%4), 4*(c%4)+4).
Each core computes its 4 heads' attention and a partial output projection
(row-sharded Wout); the host sums the 4 partials per batch and adds bout.

All matmul operands bf16 (fp32 PSUM accumulation); end-to-end rel err ~4e-3.
Scores computed transposed ([k x q]) so softmax needs no PE transposes; a
ones column in V makes attn@V accumulate the softmax denominator; exp on
ScalarE from PSUM, no max-subtraction; causal mask via gpsimd affine_select;
PSUM: scores [128,1024]x2 + av [65,512]x4 = 8 banks (deep av buffering keeps
the PE gap-free so the HAM clock gate stays warm).
"""

import os
from contextlib import ExitStack

import ml_dtypes
import numpy as np

import concourse.bass as bass
import concourse.mybir as mybir
import concourse.tile as tile
from concourse import bacc
from concourse.bass_utils import run_bass_kernel_spmd

f32 = mybir.dt.float32
bf16 = mybir.dt.bfloat16
bfnp = ml_dtypes.bfloat16

S = 2048
E = 1024
HC = 4  # heads per core
D = 64
C = HC * D  # 256 per-core head dims
NE = E // 128  # 8 contraction chunks

Exp = mybir.ActivationFunctionType.Exp
Ln = mybir.ActivationFunctionType.Ln
Ident = mybir.ActivationFunctionType.Identity


def _build_kernel(tc, qt, wq, wk, wv, wo, bq, bk, bv, y):
    nc = tc.nc
    rrow = nc.dram_tensor("rrow", [16, 512], f32).ap()
    with ExitStack() as ctx:
        const = ctx.enter_context(tc.tile_pool(name="const", bufs=1))
        qt_sb = const.tile([128, NE, S], bf16)
        wq_sb = const.tile([128, NE, C], bf16)
        wk_sb = const.tile([128, NE, C], bf16)
        wv_sb = const.tile([128, NE, C], bf16)
        wo_sb = const.tile([128, 2, E], bf16)
        bq_sb = const.tile([128, 2], f32)
        bk_sb = const.tile([128, 2], f32)
        bv_sb = const.tile([1, C], bf16)
        ones_sb = const.tile([1, 128], bf16)
        qT_sb = const.tile([128, 2, S], bf16)
        kT_sb = const.tile([128, 2, S], bf16)
        v_sb = [
            const.tile([128, HC, D + 1], bf16, tag=f"v{si}", name=f"v_sb{si}")
            for si in range(16)
        ]
        out_sb = const.tile([128, 2, S], bf16)

        # --- loads (prepacked contiguous; issue spread over engine queues) ---
        nc.sync.dma_start(bq_sb[:], bq[:])
        nc.sync.dma_start(bk_sb[:], bk[:])
        nc.sync.dma_start(bv_sb[:], bv[:])
        nc.scalar.dma_start(wq_sb[:], wq.rearrange("(p a) -> p a", p=128))
        nc.gpsimd.dma_start(wk_sb[:], wk.rearrange("(p a) -> p a", p=128))
        nc.scalar.dma_start(wv_sb[:], wv.rearrange("(p a) -> p a", p=128))
        nc.gpsimd.dma_start(wo_sb[:], wo.rearrange("(p a) -> p a", p=128))
        qt_r = qt.rearrange("(i h p s) -> i h p s", i=NE, h=2, p=64)
        _qeng = [nc.sync, nc.scalar, nc.gpsimd]
        for i in range(NE):
            for ph in range(2):
                eng = _qeng[(2 * i + ph) % 3]
                eng.dma_start(
                    qt_sb[64 * ph : 64 * ph + 64, i, :],
                    qt_r[i, ph, :, :],
                )
        nc.vector.memset(ones_sb[:], 1.0)
        for si in range(16):
            nc.gpsimd.memset(v_sb[si][:, :, D : D + 1], 1.0)

        # --- qkv projections (own psum scope, deep buffering) ---
        with tc.tile_pool(name="pqk", bufs=6, space="PSUM") as pqk:

            def qk_chunk(m):
                for g in range(4):
                    for wsb, dst, bsb, scale in (
                        (wq_sb, qT_sb, bq_sb, 0.125),
                        (wk_sb, kT_sb, bk_sb, 1.0),
                    ):
                        ps = pqk.tile(
                            [128, 512], f32, tag="mix", name=f"pqk{m}_{g}_{scale}"
                        )
                        for i in range(NE):
                            nc.tensor.matmul(
                                ps[:],
                                lhsT=wsb[:, i, 128 * m : 128 * m + 128],
                                rhs=qt_sb[:, i, 512 * g : 512 * g + 512],
                                start=(i == 0),
                                stop=(i == NE - 1),
                            )
                        nc.scalar.activation(
                            dst[:, m, 512 * g : 512 * g + 512],
                            ps[:],
                            Ident,
                            bias=bsb[:, m : m + 1],
                            scale=scale,
                        )

            qk_chunk(0)
            qk_chunk(1)
            for si in range(16):
                ps = pqk.tile([128, 512], f32, tag="mix", name=f"pv{si}")
                for i in range(NE):
                    nc.tensor.matmul(
                        ps[:, 0:C],
                        lhsT=qt_sb[:, i, 128 * si : 128 * si + 128],
                        rhs=wv_sb[:, i, :],
                        start=(i == 0),
                        stop=False,
                    )
                nc.tensor.matmul(
                    ps[:, 0:C],
                    lhsT=ones_sb[:, 0:128],
                    rhs=bv_sb[:],
                    start=False,
                    stop=True,
                )
                nc.vector.tensor_copy(
                    v_sb[si][:, :, 0:D],
                    ps[:, 0:C].rearrange("p (h d) -> p h d", h=HC),
                )

        # --- attention: sequential heads; deep av buffering kills PE gaps ---
        with tc.tile_pool(name="psc", bufs=2, space="PSUM") as psc, tc.tile_pool(
            name="pav", bufs=4, space="PSUM"
        ) as pav, tc.tile_pool(name="ppool", bufs=10) as ppool, tc.tile_pool(
            name="rl", bufs=6
        ) as rl:
            for h in range(HC):
                pr, j = h // 2, h % 2
                b0 = 64 * j
                for Hh in range(2):  # q halves of 1024
                    q0 = 1024 * Hh
                    avq = [
                        pav.tile(
                            [D + 1, 512], f32, tag="av", name=f"av{h}_{Hh}_{g2}"
                        )
                        for g2 in range(2)
                    ]
                    for kc in range(8 * Hh + 8):
                        md = kc - 8 * Hh
                        psj = psc.tile(
                            [128, 1024], f32, tag="sc", name=f"sc{h}_{Hh}_{kc}"
                        )
                        for g2 in range(2):
                            if md >= 4 and g2 == 0:
                                continue
                            nc.tensor.matmul(
                                psj[:, 512 * g2 : 512 * g2 + 512],
                                lhsT=kT_sb[
                                    b0 : b0 + 64, pr, 128 * kc : 128 * kc + 128
                                ],
                                rhs=qT_sb[
                                    b0 : b0 + 64,
                                    pr,
                                    q0 + 512 * g2 : q0 + 512 * g2 + 512,
                                ],
                                start=True,
                                stop=True,
                            )
                        pt = ppool.tile([128, 1024], bf16, tag="p")
                        e0 = max(0, 128 * md)
                        if 0 <= md <= 7:
                            zs = 0 if md < 4 else 512
                            if 128 * md > zs:
                                nc.gpsimd.memset(pt[:, zs : 128 * md], 0.0)
                        nc.scalar.activation(pt[:, e0:1024], psj[:, e0:1024], Exp)
                        if 0 <= md <= 7:
                            blk = pt[:, 128 * md : 128 * md + 128]
                            nc.gpsimd.affine_select(
                                out=blk,
                                in_=blk,
                                pattern=[[1, 128]],
                                compare_op=mybir.AluOpType.is_ge,
                                fill=0.0,
                                base=0,
                                channel_multiplier=-1,
                            )
                        for g2 in range(2):
                            if md >= 4 and g2 == 0:
                                continue
                            nc.tensor.matmul(
                                avq[g2][:],
                                lhsT=v_sb[kc][:, h, :],
                                rhs=pt[:, 512 * g2 : 512 * g2 + 512],
                                start=(kc == 0),
                                stop=(
                                    kc == (8 * Hh + 3 if g2 == 0 else 8 * Hh + 7)
                                ),
                            )
                    for g2 in range(2):
                        av = avq[g2]
                        gq = 2 * Hh + g2
                        # softmax denom l (psum row 64) -> 1/l -> broadcast
                        l_sb = rl.tile(
                            [D + 1, 512], f32, tag="l", name=f"l{h}_{gq}"
                        )
                        nc.vector.tensor_copy(
                            l_sb[D : D + 1, :], av[D : D + 1, :]
                        )
                        ltall = rl.tile(
                            [128, 4], f32, tag="ltall", name=f"lt{h}_{gq}"
                        )
                        l_row = l_sb[D : D + 1, :]
                        nc.sync.dma_start(
                            ltall[:],
                            bass.AP(
                                tensor=l_row.tensor,
                                offset=l_row.offset,
                                ap=[list(l_row.ap[0]), [4, 128], [1, 4]],
                            ),
                        )
                        nc.vector.reciprocal(ltall[:], ltall[:])
                        ridx = 4 * h + gq
                        nc.sync.dma_start(
                            rrow[ridx, :].rearrange("(p c) -> p c", p=128),
                            ltall[:],
                        )
                        rb = rl.tile([64, 512], f32, tag="rb", name=f"rb{h}_{gq}")
                        rr = rrow[ridx, :]
                        nc.sync.dma_start(
                            rb[:],
                            bass.AP(
                                tensor=rr.tensor,
                                offset=rr.offset,
                                ap=[[0, 64], [1, 512]],
                            ),
                        )
                        nc.vector.tensor_mul(
                            out_sb[b0 : b0 + 64, pr, 512 * gq : 512 * gq + 512],
                            av[0:D, :],
                            rb[:],
                        )

        # --- output projection (partial: this core's 256 contraction rows) ---
        with tc.tile_pool(name="py", bufs=4, space="PSUM") as py, tc.tile_pool(
            name="ysb", bufs=4
        ) as ysb:
            for t in range(16):
                yt = ysb.tile([128, E], f32, tag="yt", name=f"yt{t}")
                for e in range(2):
                    ps = py.tile([128, 512], f32, tag="pj", name=f"py{t}_{e}")
                    for m in range(2):
                        nc.tensor.matmul(
                            ps[:],
                            lhsT=out_sb[:, m, 128 * t : 128 * t + 128],
                            rhs=wo_sb[:, m, 512 * e : 512 * e + 512],
                            start=(m == 0),
                            stop=(m == 1),
                        )
                    if e == 0:
                        nc.vector.tensor_copy(yt[:, 0:512], ps[:])
                    else:
                        nc.scalar.copy(yt[:, 512:1024], ps[:])
                nc.sync.dma_start(y[t, :, :], yt[:])


_NC = None


def build_nc():
    global _NC
    if _NC is not None:
        return _NC
    nc = bacc.Bacc("TRN2", target_bir_lowering=False, debug=False, num_devices=8)
    qt = nc.dram_tensor("qt", [NE * 2 * 64 * S], bf16, kind="ExternalInput").ap()
    wq = nc.dram_tensor("wq", [128 * NE * C], bf16, kind="ExternalInput").ap()
    wk = nc.dram_tensor("wk", [128 * NE * C], bf16, kind="ExternalInput").ap()
    wv = nc.dram_tensor("wv", [128 * NE * C], bf16, kind="ExternalInput").ap()
    wo = nc.dram_tensor("wo", [128 * 2 * E], bf16, kind="ExternalInput").ap()
    bq = nc.dram_tensor("bq", [128, 2], f32, kind="ExternalInput").ap()
    bk = nc.dram_tensor("bk", [128, 2], f32, kind="ExternalInput").ap()
    bv = nc.dram_tensor("bv", [1, C], bf16, kind="ExternalInput").ap()
    y = nc.dram_tensor("y", [16, 128, E], f32, kind="ExternalOutput").ap()
    with tile.TileContext(nc) as tc:
        _build_kernel(tc, qt, wq, wk, wv, wo, bq, bk, bv, y)
    nc.compile()
    _NC = nc
    return nc


def make_in_maps(Q, Wqkv, bqkv, Wout):
    """Per-core input dicts (8 cores: batch-major, then head-group)."""
    in_maps = []
    for c in range(8):
        b, hq = c // 4, c % 4
        cs = C * hq
        qt_np = np.ascontiguousarray(
            Q[b].T.reshape(NE, 2, 64, S)
        ).astype(bfnp).reshape(-1)

        def packw(w):
            # [E, C] -> sbuf layout [128 p, NE, C] flattened
            return (
                np.ascontiguousarray(
                    w.reshape(NE, 128, C).transpose(1, 0, 2)
                )
                .astype(bfnp)
                .reshape(-1)
            )

        wq_np = packw(Wqkv[:, cs : cs + C])
        wk_np = packw(Wqkv[:, E + cs : E + cs + C])
        wv_np = packw(Wqkv[:, 2 * E + cs : 2 * E + cs + C])
        bq_np = np.ascontiguousarray(
            (bqkv[cs : cs + C].astype(np.float32) * 0.125).reshape(2, 128).T
        )
        bk_np = np.ascontiguousarray(
            bqkv[E + cs : E + cs + C].astype(np.float32).reshape(2, 128).T
        )
        bv_np = bqkv[2 * E + cs : 2 * E + cs + C].reshape(1, C).astype(bfnp)
        wo_np = (
            np.ascontiguousarray(
                Wout[cs : cs + C, :].reshape(2, 128, E).transpose(1, 0, 2)
            )
            .astype(bfnp)
            .reshape(-1)
        )
        in_maps.append(
            {
                "qt": qt_np,
                "wo": wo_np,
                "wq": wq_np,
                "wk": wk_np,
                "wv": wv_np,
                "bq": bq_np,
                "bk": bk_np,
                "bv": bv_np,
            }
        )
    return in_maps


def kernel(Q, Wqkv, bqkv, Wout, bout, _trace=False, _trace_kwargs=None):
    Q = np.asarray(Q, dtype=np.float32)
    Wqkv = np.asarray(Wqkv, dtype=np.float32)
    bqkv = np.asarray(bqkv, dtype=np.float32)
    Wout = np.asarray(Wout, dtype=np.float32)
    bout = np.asarray(bout, dtype=np.float32)

    nc = build_nc()
    in_maps = make_in_maps(Q, Wqkv, bqkv, Wout)

    kwargs = {}
    if _trace:
        kwargs = dict(trace=True, trace_cores=list(range(8)))
        if _trace_kwargs:
            kwargs.update(_trace_kwargs)
    res = run_bass_kernel_spmd(nc, in_maps, core_ids=list(range(8)), **kwargs)

    out = np.zeros((2, S, E), dtype=np.float32)
    for c in range(8):
        yc = np.asarray(res.results[c]["y"], dtype=np.float32).reshape(S, E)
        out[c // 4] += yc
    out += bout.astype(np.float32)[None, None, :]
    if _trace:
        kernel._last_results = res
    return out



# revision 3
# speedup vs baseline: 1.0823x; 1.0823x over previous
"""Multi-head self-attention (B=2, S=2048, E=1024, H=16, causal) on 8 trn2 cores.

Sharding: core c handles batch b = c // 4 and heads [4*(c%4), 4*(c%4)+4).
Each core computes its 4 heads' attention and a partial output projection
(row-sharded Wout); the host sums the 4 partials per batch and adds bout.

Key optimizations over the naive version:
- qt loaded S-major in four 1MB blocks so the first projection matmuls start
  ~6us in instead of waiting for the full 4MB.
- Attention processes head PAIRS: the two heads of a pair live in SBUF
  partitions 0-63 / 64-127, so their K=64 score matmuls land on different
  PE row-groups (tile_position (0,0) vs (64,0)) and stream concurrently.
- exp is split across ScalarE (exact activation) and VectorE (Schraudolph
  fast-exp: bits = round(x*128/ln2 + B) written as int16 = bf16 bits).
- V stationary padded to 128 columns (ones in 64..127) so LDWEIGHTS runs
  with FWL; column 64 doubles as the softmax-denominator ones column.
- Output projection partials stored/DMAd as bf16 (halves output traffic).
"""

import os
from contextlib import ExitStack

import ml_dtypes
import numpy as np

import concourse.bass as bass
import concourse.mybir as mybir
import concourse.tile as tile
from concourse import bacc
from concourse.bass_utils import run_bass_kernel_spmd

f32 = mybir.dt.float32
bf16 = mybir.dt.bfloat16
i16 = mybir.dt.int16
bfnp = ml_dtypes.bfloat16

S = 2048
E = 1024
HC = 4  # heads per core
D = 64
C = HC * D  # 256 per-core head dims
NE = E // 128  # 8 contraction chunks

Exp = mybir.ActivationFunctionType.Exp
Ident = mybir.ActivationFunctionType.Identity
Mult = mybir.AluOpType.mult
Add = mybir.AluOpType.add

# Schraudolph fast-exp constants in bf16-bit domain
A16 = 128.0 / np.log(2.0)
B16C = 127.0 * 128.0 - 7.0


def _build_kernel(tc, qt, wq, wk, wv, wo, bq, bk, bv, y):
    nc = tc.nc
    rrow = nc.dram_tensor("rrow", [16, 512], f32).ap()
    with ExitStack() as ctx:
        const = ctx.enter_context(tc.tile_pool(name="const", bufs=1))
        qt_sb = const.tile([128, NE, S], bf16)
        wq_sb = const.tile([128, NE, C], bf16)
        wk_sb = const.tile([128, NE, C], bf16)
        wv_sb = const.tile([128, NE, C], bf16)
        wo_sb = const.tile([128, 2, E], bf16)
        bq_sb = const.tile([128, 2], f32)
        bk_sb = const.tile([128, 2], f32)
        bv_sb = const.tile([1, C], bf16)
        ones_sb = const.tile([1, 128], bf16)
        qT_sb = const.tile([128, 2, S], bf16)
        kT_sb = const.tile([128, 2, S], bf16)
        v_sb = [
            const.tile([128, HC, 128], bf16, tag=f"v{si}", name=f"v_sb{si}")
            for si in range(16)
        ]
        out_sb = const.tile([128, 2, S], bf16)

        # --- loads: weights first (parallel queues), qt S-major on sync ---
        nc.scalar.dma_start(wq_sb[:], wq.rearrange("(p a) -> p a", p=128))
        nc.gpsimd.dma_start(wk_sb[:], wk.rearrange("(p a) -> p a", p=128))
        nc.sync.dma_start(bq_sb[:], bq[:])
        nc.sync.dma_start(bk_sb[:], bk[:])
        nc.sync.dma_start(bv_sb[:], bv[:])
        qt_r = qt.rearrange("(g p i c) -> g p i c", g=4, p=128, i=NE)
        for g in range(4):
            nc.sync.dma_start(
                qt_sb[:, :, 512 * g : 512 * g + 512], qt_r[g]
            )
        nc.scalar.dma_start(wv_sb[:], wv.rearrange("(p a) -> p a", p=128))
        nc.gpsimd.dma_start(wo_sb[:], wo.rearrange("(p a) -> p a", p=128))
        nc.vector.memset(ones_sb[:], 1.0)
        for si in range(16):
            nc.gpsimd.memset(v_sb[si][:, :, D:128], 1.0)

        # --- qkv projections, g-block pipelined ---
        with tc.tile_pool(name="pqk", bufs=6, space="PSUM") as pqk:

            def qk_g(g):
                for m in range(2):
                    for wsb, dst, bsb in (
                        (wq_sb, qT_sb, bq_sb),
                        (wk_sb, kT_sb, bk_sb),
                    ):
                        ps = pqk.tile(
                            [128, 512], f32, tag="mix", name=f"pqk{g}_{m}_{id(wsb)}"
                        )
                        for i in range(NE):
                            nc.tensor.matmul(
                                ps[:],
                                lhsT=wsb[:, i, 128 * m : 128 * m + 128],
                                rhs=qt_sb[:, i, 512 * g : 512 * g + 512],
                                start=(i == 0),
                                stop=(i == NE - 1),
                            )
                        nc.scalar.activation(
                            dst[:, m, 512 * g : 512 * g + 512],
                            ps[:],
                            Ident,
                            bias=bsb[:, m : m + 1],
                            scale=1.0,
                        )

            def v_si(si):
                ps = pqk.tile([128, 512], f32, tag="mix", name=f"pv{si}")
                for i in range(NE):
                    nc.tensor.matmul(
                        ps[:, 0:C],
                        lhsT=qt_sb[:, i, 128 * si : 128 * si + 128],
                        rhs=wv_sb[:, i, :],
                        start=(i == 0),
                        stop=False,
                    )
                nc.tensor.matmul(
                    ps[:, 0:C],
                    lhsT=ones_sb[:, 0:128],
                    rhs=bv_sb[:],
                    start=False,
                    stop=True,
                )
                nc.vector.tensor_copy(
                    v_sb[si][:, :, 0:D],
                    ps[:, 0:C].rearrange("p (h d) -> p h d", h=HC),
                )

            for g in range(4):
                qk_g(g)
                for si in range(4 * g, 4 * g + 4):
                    v_si(si)

        # --- attention: head pairs, row-tiled concurrent scores ---
        unit_idx = [0]
        with tc.tile_pool(name="psc", bufs=4, space="PSUM") as psc, tc.tile_pool(
            name="pav", bufs=4, space="PSUM"
        ) as pav, tc.tile_pool(name="ppool", bufs=10) as ppool, tc.tile_pool(
            name="rl", bufs=8
        ) as rl:

            def post_g2(pr, hb, av, gq):
                # softmax denominator -> reciprocal -> broadcast -> normalize
                h = 2 * pr + hb
                l_sb = rl.tile([1, 512], f32, tag="l", name=f"l{h}_{gq}")
                nc.vector.tensor_copy(l_sb[:], av[D : D + 1, :])
                ltall = rl.tile([128, 4], f32, tag="lt", name=f"lt{h}_{gq}")
                l_row = l_sb[0:1, :]
                nc.sync.dma_start(
                    ltall[:],
                    bass.AP(
                        tensor=l_row.tensor,
                        offset=l_row.offset,
                        ap=[list(l_row.ap[0]), [4, 128], [1, 4]],
                    ),
                )
                nc.vector.reciprocal(ltall[:], ltall[:])
                ridx = 4 * h + gq
                nc.sync.dma_start(
                    rrow[ridx, :].rearrange("(p c) -> p c", p=128), ltall[:]
                )
                rb = rl.tile([64, 512], f32, tag="rb", name=f"rb{h}_{gq}")
                rr = rrow[ridx, :]
                nc.sync.dma_start(
                    rb[:],
                    bass.AP(
                        tensor=rr.tensor,
                        offset=rr.offset,
                        ap=[[0, 64], [1, 512]],
                    ),
                )
                b0 = 64 * hb
                nc.vector.tensor_mul(
                    out_sb[b0 : b0 + 64, pr, 512 * gq : 512 * gq + 512],
                    av[0:D, :],
                    rb[:],
                )

            for pr in range(2):
                for Hh in range(2):
                    q0 = 1024 * Hh
                    klast = 8 * Hh + 8
                    # av accumulators: [head(2)][g2(2)]
                    avt = [
                        [
                            pav.tile(
                                [128, 512],
                                f32,
                                tag="av",
                                name=f"av{pr}_{Hh}_{hb}_{g2}",
                            )
                            for g2 in range(2)
                        ]
                        for hb in range(2)
                    ]
                    for kc in range(klast):
                        md = kc - 8 * Hh
                        for g2 in range(2):
                            if md >= 4 and g2 == 0:
                                continue
                            qc0 = q0 + 512 * g2
                            # diagonal block bookkeeping (local cols in unit)
                            diag = 0 <= md <= 7 and g2 == md // 4
                            ds = 128 * md - 512 * g2 if diag else 0
                            pss = []
                            pts = []
                            for hb in range(2):
                                ps = psc.tile(
                                    [128, 512],
                                    f32,
                                    tag="sc",
                                    name=f"sc{pr}_{Hh}_{kc}_{g2}_{hb}",
                                )
                                pss.append(ps)
                            # concurrent row-tiled score matmuls (adjacent)
                            for hb in range(2):
                                b0 = 64 * hb
                                nc.tensor.matmul(
                                    pss[hb][:],
                                    lhsT=kT_sb[
                                        b0 : b0 + 64,
                                        pr,
                                        128 * kc : 128 * kc + 128,
                                    ],
                                    rhs=qT_sb[b0 : b0 + 64, pr, qc0 : qc0 + 512],
                                    start=True,
                                    stop=True,
                                )
                            for hb in range(2):
                                pt = ppool.tile(
                                    [128, 512],
                                    bf16,
                                    tag="p",
                                    name=f"pt{pr}_{Hh}_{kc}_{g2}_{hb}",
                                )
                                pts.append(pt)
                                if diag and ds > 0:
                                    nc.gpsimd.memset(pt[:, 0:ds], 0.0)
                                # engine split: alternate scalar/vector per
                                # unit; every 4th unit both on scalar
                                u = unit_idx[0]
                                if u % 4 == 3:
                                    eng = "s"
                                else:
                                    eng = "s" if (u + hb) % 2 == 0 else "d"
                                if eng == "s":
                                    nc.scalar.activation(
                                        pt[:, ds:512], pss[hb][:, ds:512], Exp
                                    )
                                else:
                                    nc.vector.tensor_scalar(
                                        pt[:, ds:512].bitcast(i16),
                                        pss[hb][:, ds:512],
                                        A16,
                                        B16C,
                                        Mult,
                                        Add,
                                    )
                                if diag:
                                    blk = pt[:, ds : ds + 128]
                                    nc.gpsimd.affine_select(
                                        out=blk,
                                        in_=blk,
                                        pattern=[[1, 128]],
                                        compare_op=mybir.AluOpType.is_ge,
                                        fill=0.0,
                                        base=0,
                                        channel_multiplier=-1,
                                    )
                            unit_idx[0] += 1
                            stop_kc = 8 * Hh + (3 if g2 == 0 else 7)
                            for hb in range(2):
                                nc.tensor.matmul(
                                    avt[hb][g2][:],
                                    lhsT=v_sb[kc][:, 2 * pr + hb, :],
                                    rhs=pts[hb][:],
                                    start=(kc == 0),
                                    stop=(kc == stop_kc),
                                )
                                if kc == stop_kc:
                                    post_g2(
                                        pr, hb, avt[hb][g2], 2 * Hh + g2
                                    )

        # --- output projection (partial: this core's 256 contraction rows) ---
        with tc.tile_pool(name="py", bufs=4, space="PSUM") as py, tc.tile_pool(
            name="ysb", bufs=4
        ) as ysb:
            for t in range(16):
                yt = ysb.tile([128, E], bf16, tag="yt", name=f"yt{t}")
                for e in range(2):
                    ps = py.tile([128, 512], f32, tag="pj", name=f"py{t}_{e}")
                    for m in range(2):
                        nc.tensor.matmul(
                            ps[:],
                            lhsT=out_sb[:, m, 128 * t : 128 * t + 128],
                            rhs=wo_sb[:, m, 512 * e : 512 * e + 512],
                            start=(m == 0),
                            stop=(m == 1),
                        )
                    if e == 0:
                        nc.vector.tensor_copy(yt[:, 0:512], ps[:])
                    else:
                        nc.scalar.copy(yt[:, 512:1024], ps[:])
                nc.sync.dma_start(y[t, :, :], yt[:])


_NC = None


def build_nc():
    global _NC
    if _NC is not None:
        return _NC
    nc = bacc.Bacc("TRN2", target_bir_lowering=False, debug=False, num_devices=8)
    qt = nc.dram_tensor("qt", [4 * 128 * NE * 512], bf16, kind="ExternalInput").ap()
    wq = nc.dram_tensor("wq", [128 * NE * C], bf16, kind="ExternalInput").ap()
    wk = nc.dram_tensor("wk", [128 * NE * C], bf16, kind="ExternalInput").ap()
    wv = nc.dram_tensor("wv", [128 * NE * C], bf16, kind="ExternalInput").ap()
    wo = nc.dram_tensor("wo", [128 * 2 * E], bf16, kind="ExternalInput").ap()
    bq = nc.dram_tensor("bq", [128, 2], f32, kind="ExternalInput").ap()
    bk = nc.dram_tensor("bk", [128, 2], f32, kind="ExternalInput").ap()
    bv = nc.dram_tensor("bv", [1, C], bf16, kind="ExternalInput").ap()
    y = nc.dram_tensor("y", [16, 128, E], bf16, kind="ExternalOutput").ap()
    with tile.TileContext(nc) as tc:
        _build_kernel(tc, qt, wq, wk, wv, wo, bq, bk, bv, y)
    nc.compile()
    _NC = nc
    return nc


def make_in_maps(Q, Wqkv, bqkv, Wout):
    """Per-core input dicts (8 cores: batch-major, then head-group)."""
    in_maps = []
    for c in range(8):
        b, hq = c // 4, c % 4
        cs = C * hq
        # qt: S-major g-blocks [4, 128, NE, 512]
        qtt = np.ascontiguousarray(Q[b].T).reshape(NE, 128, S)
        qt_np = (
            np.stack(
                [
                    np.ascontiguousarray(
                        qtt[:, :, 512 * g : 512 * g + 512].transpose(1, 0, 2)
                    )
                    for g in range(4)
                ]
            )
            .astype(bfnp)
            .reshape(-1)
        )

        def packw(w):
            # [E, C] -> sbuf layout [128 p, NE, C] flattened
            return (
                np.ascontiguousarray(w.reshape(NE, 128, C).transpose(1, 0, 2))
                .astype(bfnp)
                .reshape(-1)
            )

        wq_np = packw(Wqkv[:, cs : cs + C] * 0.125)
        wk_np = packw(Wqkv[:, E + cs : E + cs + C])
        wv_np = packw(Wqkv[:, 2 * E + cs : 2 * E + cs + C])
        bq_np = np.ascontiguousarray(
            (bqkv[cs : cs + C].astype(np.float32) * 0.125).reshape(2, 128).T
        )
        bk_np = np.ascontiguousarray(
            bqkv[E + cs : E + cs + C].astype(np.float32).reshape(2, 128).T
        )
        bv_np = bqkv[2 * E + cs : 2 * E + cs + C].reshape(1, C).astype(bfnp)
        wo_np = (
            np.ascontiguousarray(
                Wout[cs : cs + C, :].reshape(2, 128, E).transpose(1, 0, 2)
            )
            .astype(bfnp)
            .reshape(-1)
        )
        in_maps.append(
            {
                "qt": qt_np,
                "wo": wo_np,
                "wq": wq_np,
                "wk": wk_np,
                "wv": wv_np,
                "bq": bq_np,
                "bk": bk_np,
                "bv": bv_np,
            }
        )
    return in_maps


def kernel(Q, Wqkv, bqkv, Wout, bout, _trace=False, _trace_kwargs=None):
    Q = np.asarray(Q, dtype=np.float32)
    Wqkv = np.asarray(Wqkv, dtype=np.float32)
    bqkv = np.asarray(bqkv, dtype=np.float32)
    Wout = np.asarray(Wout, dtype=np.float32)
    bout = np.asarray(bout, dtype=np.float32)

    nc = build_nc()
    in_maps = make_in_maps(Q, Wqkv, bqkv, Wout)

    kwargs = {}
    if _trace:
        kwargs = dict(trace=True, trace_cores=list(range(8)))
        if _trace_kwargs:
            kwargs.update(_trace_kwargs)
    res = run_bass_kernel_spmd(nc, in_maps, core_ids=list(range(8)), **kwargs)

    out = np.zeros((2, S, E), dtype=np.float32)
    for c in range(8):
        yc = np.asarray(res.results[c]["y"]).astype(np.float32).reshape(S, E)
        out[c // 4] += yc
    out += bout.astype(np.float32)[None, None, :]
    if _trace:
        kernel._last_results = res
    return out
